# revision 14
# baseline (speedup 1.0000x reference)
"""Trainium2 Bass kernel for nn_LocalizeAttention (27-point 3D neighbourhood gather).

out[b,h,(pi,pj,pk),(i,j,k),d] = x[b,h,(pi+i-1, pj+j-1, pk+k-1),d], zero outside.

Strategy (per core, SPMD over 8 cores; 2 (b,h) volumes per core):
  - host zero-pads each volume to [26,26,26,32] so boundary handling is free
  - 3 "slab" loads per volume, pre-shifted along pi (partition dim carries
    (pi, pj_outer); shifts along the partition dim can't be done by compute
    engines, so each di' gets its own HBM load)
  - slab free dim holds (pj 8-wide with halo, pk_padded 26, d 32): the dj'/dk'
    shifts are pure free-dim offsets
  - 9 shifted copies per output tile (di' x dj'; the 3 dk' merge into one
    contiguous 96-float run) assemble [96 part, (pj_i 6, pk_l 2, s 27, d 32)]
    tiles, split across Vector/GpSimd/Scalar engines
  - dense 4MB SBUF->HBM stores
"""

import numpy as np

B, HEADS, DH = 2, 8, 32
H = W = D = 24
N = H * W * D
FN = 27
NCORES = 8
NVOL = (B * HEADS) // NCORES  # 2 volumes per core


def _build_nc(nvol, h, w, d, dh, pjo, pji, pkb):
    import concourse.bass as bass
    import concourse.mybir as mybir
    from concourse.ap import AP
    from concourse.bacc import Bacc
    from concourse.tile import TileContext

    f32 = mybir.dt.float32
    hp, wp, dp = h + 2, w + 2, d + 2
    pjh = pji + 2                      # pj window incl. halo
    P = h * pjo                        # partitions
    assert pjo * pji == w and P <= 128
    fn = 27
    n = h * w * d
    s_kp = dh                          # x3pad strides (elements)
    s_jp = dp * dh
    s_ip = wp * dp * dh
    vol_pad = hp * wp * dp * dh
    slab_f = pjh * dp * dh             # slab free size
    out_f = pji * pkb * fn * dh        # out-tile free size
    ntiles = d // pkb
    vol_out = n * fn * dh
    run = 3 * dh                       # merged (dk', d) run

    nc = Bacc()
    xpad = nc.declare_dram_parameter("xpad", [nvol, hp, wp, dp, dh], f32,
                                     isOutput=False)
    out = nc.declare_dram_parameter("out", [nvol, n, fn, dh], f32,
                                    isOutput=True)
    xt = xpad[:].tensor
    ot = out[:].tensor

    import contextlib
    with contextlib.ExitStack() as ctx:
        tc = ctx.enter_context(TileContext(nc))
        slabs = [ctx.enter_context(nc.sbuf_tensor(f"slab{i}", [P, slab_f], f32))
                 for i in range(3)]
        otiles = [ctx.enter_context(nc.sbuf_tensor(f"otile{i}", [P, out_f], f32))
                  for i in range(2)]
        scratch = ctx.enter_context(nc.sbuf_tensor("scratch", [P, 6], f32))
        # two copy engines, rotated per tile (matches the 2 otile buffers)
        engines = [nc.vector, nc.gpsimd]
        tix = 0
        for v in range(nvol):
            for dip in range(3):
                src = AP(xt, v * vol_pad + dip * s_ip,
                         [[s_ip, h], [pji * s_jp, pjo], [1, slab_f]])
                nc.sync.dma_start(out=slabs[dip][:], in_=src)
            # wait-absorbers: soak up the per-slab DMA-lane waits on each
            # copy engine so no real copy needs >2 sync waits (HW cap)
            for ei, eng in enumerate(engines):
                for dip in range(3):
                    col = ei * 3 + dip
                    eng.tensor_copy(out=scratch[:, col:col + 1],
                                    in_=slabs[dip][:, 0:1])
            for t in range(ntiles):
                otile = otiles[tix % 2]
                # one engine per tile (stores then wait on a single sem;
                # engines run different tiles concurrently)
                eng = engines[tix % 2]
                tix += 1
                for dip in range(3):
                    slab = slabs[dip]
                    sbase = slab[:]
                    obase = otile[:]
                    for djp in range(3):
                        csrc = AP(sbase.tensor,
                                  sbase.offset + djp * s_jp + t * pkb * s_kp,
                                  [[slab_f, P], [s_jp, pji], [s_kp, pkb],
                                   [1, run]])
                        cdst = AP(obase.tensor,
                                  obase.offset + (dip * 9 + djp * 3) * dh,
                                  [[out_f, P], [pkb * fn * dh, pji],
                                   [fn * dh, pkb], [1, run]])
                        if hasattr(eng, "tensor_copy"):
                            eng.tensor_copy(out=cdst, in_=csrc)
                        else:
                            eng.copy(out=cdst, in_=csrc)
                sdst = AP(ot, v * vol_out + t * pkb * fn * dh,
                          [[pji * w * fn * dh, P],
                           [w * fn * dh, pji], [1, pkb * fn * dh]])
                nc.sync.dma_start(out=sdst, in_=otile[:])
    nc.finalize()
    return nc


def _pad_volumes(x):
    # x: [nvol, N, dh] -> [nvol, hp, wp, dp, dh] zero-padded
    nvol = x.shape[0]
    xv = x.reshape(nvol, H, W, D, DH)
    xp = np.zeros((nvol, H + 2, W + 2, D + 2, DH), dtype=np.float32)
    xp[:, 1:H + 1, 1:W + 1, 1:D + 1, :] = xv
    return xp


def _run(x, trace=False):
    from concourse.bass_utils import run_bass_kernel_spmd

    x = np.asarray(x, dtype=np.float32)
    assert x.shape == (B, HEADS, N, DH), x.shape
    xf = x.reshape(B * HEADS, N, DH)
    nc = _build_nc(NVOL, H, W, D, DH, 4, 6, 2)
    in_maps = [{"xpad": _pad_volumes(xf[i * NVOL:(i + 1) * NVOL])}
               for i in range(NCORES)]
    res = run_bass_kernel_spmd(nc, in_maps, list(range(NCORES)), trace=trace)
    outs = np.concatenate([res.results[i]["out"] for i in range(NCORES)],
                          axis=0)
    return outs.reshape(B, HEADS, N, FN, DH), res


def kernel(x, height, width, depth, **_):
    assert int(height) == H and int(width) == W and int(depth) == D
    out, _res = _run(x, trace=False)
    return out


def kernel_profiled(x):
    out, res = _run(x, trace=True)
    return out, res
